# revision 35
# baseline (speedup 1.0000x reference)
"""MoE (top-2 routed GluMLP) Trainium2 kernel, expert-parallel over 8 NeuronCores.

Contract: kernel(**inputs) takes the FULL unsharded inputs
  x  [2, 2048, 1024] f32
  Wr [8, 1024] f32           router
  Wg [8, 4096, 1024] f32     gate proj per expert
  Wu [8, 4096, 1024] f32     up proj per expert
  Wd [8, 1024, 4096] f32     down proj per expert
and returns the FULL output [2, 2048, 1024] f32.

Strategy (expert-parallel, sparse dispatch):
  - Routing (softmax + top-2 + renormalize) is computed on host with jax on
    CPU using the exact ops of the reference, so the selected expert set and
    combine weights match the reference bit-for-bit.
  - Tokens are gathered per expert (capacity = max expert load, rounded to 8)
    and dispatched to the core owning that expert.
  - Each core runs a weighted GluMLP over its Tc tokens:
        out[t, :] = w[t] * (relu(x_t @ Wg_e.T) * (x_t @ Wu_e.T)) @ Wd_e.T
    with matmul operands in fp16 (same 10-bit mantissa as TF32) and fp32
    accumulation in PSUM; host converts operands so rounding is exact.
  - Host scatter-adds the per-core outputs back into the full [T, D] output.

Env: MOE_MM_DT selects matmul operand dtype:
  f16 (default): fp16 operands (same 10-bit mantissa as TF32), fp32 accumulate.
                 Fast weight load + half DMA. ~5e-4 rel err.
  f32r:          TF32. Same accuracy, slower weight loads, 2x DMA.
  f32:           plain fp32 matmuls (4x slower), ~1e-6 rel err.
"""

import math
import os
from contextlib import ExitStack

import numpy as np

import concourse.bass as bass
import concourse.tile as tile
from concourse import bacc, mybir
from concourse.bass_utils import run_bass_kernel_spmd

B, L, D, E, TOPK, DFF = 2, 2048, 1024, 8, 2, 4096
T = B * L
NCORES = 8
P = 128
NB = 512          # matmul moving-operand block (fp32 max; one PSUM bank of fp32 out)
DC = D // P       # 8 contraction chunks over D
FM = DFF // P     # 32 chunks over DFF

F32 = mybir.dt.float32
F32R = mybir.dt.float32r
F16 = mybir.dt.float16

# Set to True (e.g. from test.py) to run with NTFF tracing and print HW time.
PROFILE = False
TRACE_CORES = None  # e.g. list(range(8)) to profile every core
LAST_EXEC_NS = None
# Matmul dtype for the big GluMLP matmuls.
MM_DT = {"f32": F32, "f32r": F32R, "f16": F16}[os.environ.get("MOE_MM_DT", "f16")]
# Token chunk held in SBUF (h_all is [128, 32, TCH] in MM_DT).
TCH = 2048 if MM_DT is F16 else 1024


def _nblocks(tch):
    """Moving-dim blocks <=512, balanced: per-matmul cost is
    max(stream_cols/2.4GHz, ~100ns weight-load floor), so equal blocks beat
    512s-plus-tiny-tail (a tail below ~233 cols is pure LDW overhead)."""
    k = max(1, math.ceil(tch / NB))
    base, rem = divmod(tch, k)
    out, t = [], 0
    for i in range(k):
        nb = base + (1 if i < rem else 0)
        out.append((t, nb))
        t += nb
    return out


def _mgroups(ntile, gmax=8):
    """Token-subtile groups of up to gmax (PSUM-bank limited), balanced so no
    group is tiny (a group of 1 can't hide its Wd stream behind matmuls)."""
    ngroups = max(1, math.ceil(ntile / gmax))
    base, rem = divmod(ntile, ngroups)
    out, m = [], 0
    for i in range(ngroups):
        g = base + (1 if i < rem else 0)
        out.append((m, g))
        m += g
    return out


def _build_nc(Tc: int):
    """Build the single-core Bass program (SPMD: all cores run the same NEFF)."""
    nc = bacc.Bacc(
        "TRN2",
        target_bir_lowering=False,
        debug=False,
        enable_asserts=False,
        num_devices=NCORES,
    )
    mdt = MM_DT
    x_d = nc.dram_tensor("x", [P, DC, Tc], mdt, kind="ExternalInput").ap()
    w_d = nc.dram_tensor("w", [math.ceil(Tc / P), P], F32, kind="ExternalInput").ap()
    wg_d = nc.dram_tensor("wg", [FM, P, DC, P], mdt, kind="ExternalInput").ap()
    wu_d = nc.dram_tensor("wu", [FM, P, DC, P], mdt, kind="ExternalInput").ap()
    wd_d = nc.dram_tensor("wd", [FM, P, D], mdt, kind="ExternalInput").ap()
    out_d = nc.dram_tensor("out", [Tc, D], F32, kind="ExternalOutput").ap()

    with tile.TileContext(nc) as tc:
        with ExitStack() as ctx:
            _moe_body(ctx, tc, x_d, w_d, wg_d, wu_d, wd_d, out_d, Tc)
    nc.compile()
    return nc


def _moe_body(ctx, tc, x_d, w_d, wg_d, wu_d, wd_d, out_d, Tc):
    nc = tc.nc
    mdt = MM_DT
    nchunk = math.ceil(Tc / TCH)
    tca = min(TCH, Tc)  # allocated chunk width (don't waste SBUF below the cap)

    xpool = ctx.enter_context(tc.tile_pool(name="xpool", bufs=1))
    hpool = ctx.enter_context(tc.tile_pool(name="hpool", bufs=1))
    wgupool = ctx.enter_context(tc.tile_pool(name="wgupool", bufs=2))
    wdpool = ctx.enter_context(tc.tile_pool(name="wdpool", bufs=6))
    gpool = ctx.enter_context(tc.tile_pool(name="gpool", bufs=3))
    opool = ctx.enter_context(tc.tile_pool(name="opool", bufs=6))
    wtpool = ctx.enter_context(tc.tile_pool(name="wtpool", bufs=2))
    # One shared PSUM pool: phase B (ps_g/ps_u) and phase C (output groups)
    # don't overlap in time, so both get all 8 banks.
    psP = ctx.enter_context(tc.tile_pool(name="psP", bufs=8, space="PSUM"))

    for ci in range(nchunk):
        t0 = ci * TCH
        tch = min(TCH, Tc - t0)
        ntile = math.ceil(tch / P)   # token subtiles of <=128 (last may be partial)

        # Prefetch the first weight tiles ahead of the x stream so phase B can
        # start the moment the last x stripe lands (they otherwise queue
        # behind the x slices on the same DMA queues).
        pre = []
        for fm in range(2):
            wg_p = wgupool.tile([P, DC, P], mdt, tag="wg", name=f"wg_pre{fm}")
            nc.sync.dma_start(out=wg_p, in_=wg_d[fm])
            wu_p = wgupool.tile([P, DC, P], mdt, tag="wu", name=f"wu_pre{fm}")
            nc.scalar.dma_start(out=wu_p, in_=wu_d[fm])
            pre.append((wg_p, wu_p))

        # x load split across the three DMA-capable engines (sync/scalar HWDGE
        # + gpsimd SWDGE) — a single queue tops out well below HBM rate.
        # sync/scalar already carry the prefetched weight tiles, so gpsimd
        # takes 4 of the 8 stripes to equalize bytes per queue (the last
        # stripe's arrival gates phase B's first accumulation group).
        x_sb = xpool.tile([P, DC, tca], mdt, tag="x")
        dma_engines = [
            nc.sync, nc.scalar, nc.gpsimd, nc.gpsimd,
            nc.gpsimd, nc.sync, nc.scalar, nc.gpsimd,
        ]
        for dc in range(DC):
            dma_engines[dc].dma_start(
                out=x_sb[:, dc, :tch], in_=x_d[:, dc, t0 : t0 + tch]
            )
        w_sb = wtpool.tile([P, math.ceil(tca / P)], F32, tag="w")
        nc.gpsimd.dma_start(
            out=w_sb[:, :ntile],
            in_=w_d[t0 // P : t0 // P + ntile, :].rearrange("n p -> p n"),
        )

        h_all = hpool.tile([P, FM, tca], mdt, tag="h")

        # Phase B: h[f, t] = relu(g) * u for this token chunk, f-major layout.
        for fm in range(FM):
            if fm < len(pre):
                wg_sb, wu_sb = pre[fm]
            else:
                wg_sb = wgupool.tile([P, DC, P], mdt, tag="wg")
                nc.sync.dma_start(out=wg_sb, in_=wg_d[fm])
                wu_sb = wgupool.tile([P, DC, P], mdt, tag="wu")
                nc.scalar.dma_start(out=wu_sb, in_=wu_d[fm])
            for nb0, nbl in _nblocks(tch):
                ts = slice(nb0, nb0 + nbl)
                ps_g = psP.tile([P, NB], F32, tag="ps")
                ps_u = psP.tile([P, NB], F32, tag="ps")
                for dc in range(DC):
                    nc.tensor.matmul(
                        ps_g[:, :nbl],
                        lhsT=wg_sb[:, dc, :],
                        rhs=x_sb[:, dc, ts],
                        start=(dc == 0),
                        stop=(dc == DC - 1),
                    )
                for dc in range(DC):
                    nc.tensor.matmul(
                        ps_u[:, :nbl],
                        lhsT=wu_sb[:, dc, :],
                        rhs=x_sb[:, dc, ts],
                        start=(dc == 0),
                        stop=(dc == DC - 1),
                    )
                g_sb = gpool.tile([P, NB], F32, tag="g")
                nc.scalar.activation(
                    out=g_sb[:, :nbl],
                    in_=ps_g[:, :nbl],
                    func=mybir.ActivationFunctionType.Relu,
                )
                nc.vector.tensor_mul(h_all[:, fm, ts], g_sb[:, :nbl], ps_u[:, :nbl])

        # Phase C: out[t, :] = w[t] * (h.T @ WdT) for this chunk.
        # Loop dn (D half) / token groups of <=8 / fc-pairs so each Wd tile is
        # loaded once per token group (2 full Wd passes per chunk).
        for dn in range(D // NB):
            ds = slice(dn * NB, (dn + 1) * NB)
            for mg0, mgl in _mgroups(ntile):
                ps_os = []
                for j in range(mgl):
                    ps_o = psP.tile([P, NB], F32, tag="ps", name=f"ps_o{j}")
                    ps_os.append(ps_o)
                for fc2 in range(FM // 2):
                    # paired Wd loads halve the per-queue dispatch count
                    wd_sb = wdpool.tile([P, 2, NB], mdt, tag="wd")
                    eng = nc.sync if fc2 % 2 == 0 else nc.scalar
                    eng.dma_start(
                        out=wd_sb,
                        in_=wd_d[2 * fc2 : 2 * fc2 + 2, :, ds].rearrange(
                            "f p d -> p f d"
                        ),
                    )
                    for fi in range(2):
                        fc = 2 * fc2 + fi
                        for j in range(mgl):
                            mt = mg0 + j
                            pl = min(P, tch - mt * P)
                            nc.tensor.matmul(
                                ps_os[j][:pl, :],
                                lhsT=h_all[:, fc, mt * P : mt * P + pl],
                                rhs=wd_sb[:, fi, :],
                                start=(fc == 0),
                                stop=(fc == FM - 1),
                            )
                for j in range(mgl):
                    mt = mg0 + j
                    pl = min(P, tch - mt * P)
                    o_sb = opool.tile([P, NB], F32, tag="o")
                    # alternate the w[t] scaling between DVE and the otherwise
                    # idle ACT engine so group drains aren't serialized on DVE
                    if j % 2 == 0:
                        nc.vector.tensor_scalar_mul(
                            o_sb[:pl, :], ps_os[j][:pl, :], w_sb[:pl, mt : mt + 1]
                        )
                    else:
                        nc.scalar.activation(
                            out=o_sb[:pl, :],
                            in_=ps_os[j][:pl, :],
                            func=mybir.ActivationFunctionType.Copy,
                            scale=w_sb[:pl, mt : mt + 1],
                        )
                    nc.gpsimd.dma_start(
                        out=out_d[t0 + mt * P : t0 + mt * P + pl, ds], in_=o_sb[:pl, :]
                    )


_NC_CACHE: dict = {}


def _get_nc(Tc: int):
    if Tc not in _NC_CACHE:
        _NC_CACHE[Tc] = _build_nc(Tc)
    return _NC_CACHE[Tc]


def _round_tf32(a):
    """Round-to-nearest-even fp32 -> TF32 (10-bit mantissa), as np.float32."""
    u = a.astype(np.float32).view(np.uint32).astype(np.uint64)
    lsb = (u >> 13) & 1
    r = (u + 0x0FFF + lsb) & 0xFFFFE000
    return r.astype(np.uint32).view(np.float32)


def _mm_round(a):
    """Convert a host array to the dtype/value the device matmuls consume."""
    if MM_DT is F32R:
        return _round_tf32(a)
    if MM_DT is F16:
        return np.ascontiguousarray(a, dtype=np.float16)
    return np.ascontiguousarray(a, dtype=np.float32)


def _route_host(x, Wr):
    """Reference-identical routing on host (jax on CPU, same ops as reference).

    Returns (k_ids [T, K] int, k_w [T, K] f32).
    """
    import jax
    import jax.numpy as jnp

    cpu = jax.devices("cpu")[0]
    with jax.default_device(cpu):
        xt = jnp.asarray(x.reshape(T, D))
        logits = jnp.einsum("td,ed->te", xt, jnp.asarray(Wr))
        scores = jax.nn.softmax(logits, axis=-1)
        k_scores, k_ids = jax.lax.top_k(scores, TOPK)
        eps = jnp.finfo(x.dtype).eps
        k_w = k_scores / (k_scores.sum(axis=-1, keepdims=True) + eps)
        return np.asarray(k_ids), np.asarray(k_w)


def _prep_weights(Wg, Wu, Wd):
    """Per-expert weight tensors in device layouts (contiguous f32, rounded)."""
    wg_r, wu_r, wd_r = [], [], []
    for e in range(len(Wg)):
        # Wg[e]: [DFF, D]; device wants [fm, p(d_inner), dc, f_inner]
        wgt = Wg[e].T.reshape(DC, P, FM, P).transpose(2, 1, 0, 3)
        wut = Wu[e].T.reshape(DC, P, FM, P).transpose(2, 1, 0, 3)
        # Wd[e]: [D, DFF]; device wants WdT = [fc, p(f_inner), d]
        wdt = Wd[e].T.reshape(FM, P, D)
        wg_r.append(_mm_round(np.ascontiguousarray(wgt, dtype=np.float32)))
        wu_r.append(_mm_round(np.ascontiguousarray(wut, dtype=np.float32)))
        wd_r.append(_mm_round(np.ascontiguousarray(wdt, dtype=np.float32)))
    return wg_r, wu_r, wd_r


def kernel(x, Wr, Wg, Wu, Wd):
    global LAST_EXEC_NS
    x = np.asarray(x, dtype=np.float32)
    Wr = np.asarray(Wr, dtype=np.float32)
    Wg = np.asarray(Wg, dtype=np.float32)
    Wu = np.asarray(Wu, dtype=np.float32)
    Wd = np.asarray(Wd, dtype=np.float32)

    k_ids, k_w = _route_host(x, Wr)
    xt = x.reshape(T, D)

    # Gather per-expert token lists (each token appears once per selected expert).
    idx_lists, w_lists = [], []
    for e in range(E):
        tmask = k_ids == e                       # [T, K]
        tok = np.nonzero(tmask.any(axis=1))[0]   # unique tokens routed to e
        wvals = (k_w * tmask).sum(axis=1)[tok].astype(np.float32)
        idx_lists.append(tok)
        w_lists.append(wvals)

    maxload = max(len(t) for t in idx_lists)
    # Exact capacity rounded to 8 tokens (16B-aligned fp16 DMA runs); the last
    # matmul token-tile is partial (M < 128) rather than zero-padded to 128.
    Tc = max(P, ((maxload + 7) // 8) * 8)

    wg_r, wu_r, wd_r = _prep_weights(Wg, Wu, Wd)

    in_maps = []
    for e in range(E):
        tok = idx_lists[e]
        xg = np.zeros((Tc, D), dtype=np.float32)
        xg[: len(tok)] = xt[tok]
        # device layout [p(d_inner), dc, t]
        xg_r = np.ascontiguousarray(
            xg.T.reshape(DC, P, Tc).transpose(1, 0, 2), dtype=np.float32
        )
        ntile_all = math.ceil(Tc / P)
        wv = np.zeros((ntile_all * P,), dtype=np.float32)
        wv[: len(tok)] = w_lists[e]
        in_maps.append(
            {
                "x": _mm_round(xg_r),
                "w": np.ascontiguousarray(wv.reshape(ntile_all, P)),
                "wg": wg_r[e],
                "wu": wu_r[e],
                "wd": wd_r[e],
            }
        )

    nc = _get_nc(Tc)
    core_ids = list(range(NCORES))
    if PROFILE:
        res = _run_profiled(nc, in_maps, core_ids)
        LAST_EXEC_NS = res.exec_time_ns
        results = res.results
    else:
        results = run_bass_kernel_spmd(nc, in_maps, core_ids).results

    out = np.zeros((T, D), dtype=np.float32)
    for e in range(E):
        tok = idx_lists[e]
        out[tok] += results[e]["out"][: len(tok)]
    return out.reshape(B, L, D)


def _run_profiled(nc, in_maps, core_ids):
    """run_bass_kernel_spmd with trace=True, providing the NTFF hook that the
    agent image's antenv stub lacks, and skipping the artifact upload."""
    import sys
    import tempfile
    import types

    import concourse.bass_utils as bu

    if "antenv.axon_hooks" not in sys.modules:
        from trn_agent_boot.trn_boot import _ntff_profile_via_ctypes

        hook = _ntff_profile_via_ctypes("/opt/axon/libaxon_pjrt.so")
        mod = types.ModuleType("antenv.axon_hooks")
        mod.get_axon_ntff_profile_hook = lambda: hook
        mod.set_axon_ntff_profile_hook = lambda h: None
        sys.modules["antenv.axon_hooks"] = mod

    orig_upload = bu.upload_artifacts
    bu.upload_artifacts = lambda tmpdir: ""
    try:
        return run_bass_kernel_spmd(
            nc,
            in_maps,
            core_ids,
            trace=True,
            trace_cores=TRACE_CORES,
            tmpdir=tempfile.mkdtemp(prefix="moe_ntff_"),
        )
    finally:
        bu.upload_artifacts = orig_upload


if __name__ == "__main__":
    # smoke test with random data (no reference comparison)
    rng = np.random.default_rng(0)
    ins = {
        "x": rng.standard_normal((B, L, D), dtype=np.float32),
        "Wr": (rng.standard_normal((E, D)) * 0.02).astype(np.float32),
        "Wg": (rng.standard_normal((E, DFF, D)) * 0.02).astype(np.float32),
        "Wu": (rng.standard_normal((E, DFF, D)) * 0.02).astype(np.float32),
        "Wd": (rng.standard_normal((E, D, DFF)) * 0.02).astype(np.float32),
    }
    out = kernel(**ins)
    print("out", out.shape, out.dtype, float(np.abs(out).max()))
